# revision 59
# baseline (speedup 1.0000x reference)
"""Trainium2 Bass kernel for nn_Attention (B=2, S=2048, D=2048, H=16, DH=128, RoPE, causal).

Sharding: batch (2) x head-groups (4) across 8 cores. Each core computes the
partial output for 1 batch and 4 heads; the host sums the 4 head-group partials
per batch and adds b_O.

Per-core device program (all matmul operands fp16, fp32 PSUM accumulation):
  phase 1: QKV projections from host-pre-transposed X^T slabs; rotary fused into
           the PSUM->SBUF evacuation of Q^T/K^T. All inputs stream on ONE sync
           DMA ring in exact first-need order (Q heads, then K heads, then the
           one-slab-deferred V projection) so prefetches never steal SDMA
           bandwidth from the startup-critical stream.
  phase 2 (q-blocks ascending, interleaved with phase 3 by the Tile scheduler):
           per (head, q-block of 512): scores^T tiles = K_tile^T.T @ Q^T (causal
           block-skipping), exp on ACT with 1/sqrt(128) folded into the scale,
           triangular mask-mul on the diagonal 128x128 sub-block, AV accumulated
           over k-tiles. Row sums are batched: exp tiles are accumulated on the
           DVE into one fp16 tile per (head, q-block) and a SINGLE all-ones
           matmul produces the partition-replicated denominators (vs one such
           matmul per k-tile) -- saves ~9% of PE column-cycles. Fast approx
           reciprocal (custom DVE op), single normalize multiply.
  phase 3: output projection out[s,d] += Z^T[h].T @ W_O[h], fp16 partials DMAd
           out; emitted one q-block behind phase 2 so the scheduler can fill
           exp-latency PE gaps with projection matmuls.
"""

import os
import sys

if "/opt/trn_rl_repo" not in sys.path:
    sys.path.insert(0, "/opt/trn_rl_repo")

from contextlib import ExitStack

import numpy as np

import concourse.bass as bass
import concourse.tile as tile
from concourse import bacc, mybir
from concourse.bass import ds, ts
from concourse.bass_utils import run_bass_kernel_spmd

B, S, D, H, DH = 2, 2048, 2048, 16, 128
HPC = 4            # heads per core
NCORES = 8
SB = 512           # s/q block width
NSB = S // SB      # 4
NDT = D // 128     # 16 contraction d-tiles
NST = S // 128     # 16 s-tiles / k-tiles
ROT_BASE = 10000.0
SCALE = 1.0 / float(np.sqrt(float(DH)))

F16 = mybir.dt.float16
F32 = mybir.dt.float32


def _build_bass(with_bias):
    nc = bacc.Bacc()

    # --- I/O ---
    xq = nc.dram_tensor("xq", [NSB, 128, NDT * SB], F16, kind="ExternalInput")
    xk = nc.dram_tensor("xk", [NSB, 128, NDT * SB], F16, kind="ExternalInput")
    xv = nc.dram_tensor("xv", [NSB, 128, NDT * SB], F16, kind="ExternalInput")
    wq = nc.dram_tensor("wq", [128, HPC * NDT * DH], F16, kind="ExternalInput")
    wk = nc.dram_tensor("wk", [128, HPC * NDT * DH], F16, kind="ExternalInput")
    wv = nc.dram_tensor("wv", [128, NDT * HPC * DH], F16, kind="ExternalInput")
    wo = nc.dram_tensor("wo", [128, HPC * D], F16, kind="ExternalInput")
    if with_bias:
        bqf = nc.dram_tensor("bqf", [1, HPC * DH], F16, kind="ExternalInput")
        bkf = nc.dram_tensor("bkf", [1, HPC * DH], F16, kind="ExternalInput")
        bvf = nc.dram_tensor("bvf", [1, HPC * DH], F16, kind="ExternalInput")
        ones512_d = nc.dram_tensor("ones512", [1, SB], F16, kind="ExternalInput")
        ones128_d = nc.dram_tensor("ones128", [1, 128], F16, kind="ExternalInput")
    cos_d = nc.dram_tensor("cos_t", [128, S], F16, kind="ExternalInput")
    sin_d = nc.dram_tensor("sin_f", [128, S], F16, kind="ExternalInput")
    mask_d = nc.dram_tensor("mask_tri", [128, 128], F16, kind="ExternalInput")
    onesq_d = nc.dram_tensor("ones_sq", [128, 128], F16, kind="ExternalInput")
    out_part = nc.dram_tensor("out_part", [S, D], F16, kind="ExternalOutput")

    with ExitStack() as ctx:
        tc = ctx.enter_context(tile.TileContext(nc))

        glob = ctx.enter_context(tc.tile_pool(name="glob", bufs=1))
        psum = ctx.enter_context(tc.tile_pool(name="psum", bufs=1, space="PSUM"))
        persist = consts = p1w = p23 = glob

        # persistent activations
        qT = persist.tile([128, HPC * S], F16)   # (e, h*S + s)
        kT = persist.tile([128, HPC * S], F16)   # (e, h*S + s)
        vsb = persist.tile([128, NST * HPC * DH], F16)  # (s%128, stile*512 + h*128 + e)

        # constants
        mask = consts.tile([128, 128], F16)
        ones_sq = consts.tile([128, 128], F16)

        warm_sb = consts.tile([128, SB], F16)
        nc.vector.memset(warm_sb, 1.0)

        QW = NDT * DH  # per-head weight columns

        # ALL phase-1 input DMAs go on the ONE sync ring in exact first-need
        # order: a single HWDGE ring saturates HBM by itself, and FIFO order
        # means later (prefetch) transfers can never steal SDMA bandwidth
        # from the critical startup stream.
        wq_sb = p1w.tile([128, HPC * NDT * DH], F16)
        nc.sync.dma_start(wq_sb[:, 0:QW], wq[:, 0:QW])
        cosT = p1w.tile([128, S], F16)
        sinF = p1w.tile([128, S], F16)
        wv_sb = p1w.tile([128, NDT * HPC * DH], F16)

        # Preload the ACT exp table while the startup is DMA-bound (the
        # ACT_TABLE_LOAD walrus inserts before the first Exp costs ~1.5us).
        exp_warm = consts.tile([1, 16], F16)
        nc.scalar.activation(exp_warm, warm_sb[0:1, 0:16],
                             mybir.ActivationFunctionType.Exp, scale=SCALE)

        # HAM warm-up: keep the PE busy during the DMA-bound startup so the
        # first real matmuls run un-throttled (results never read).
        warm_ps = psum.tile([128, SB], F32, tag="rsum", bufs=1)
        for _ in range(21):
            nc.tensor.matmul(warm_ps, lhsT=warm_sb[:, 0:128], rhs=warm_sb,
                             start=True, stop=True)

        # phase-3 persistents (W_O DMA issued at the end of the need-ordered
        # sync chain, inside the slab block below).
        zT = p23.tile([128, HPC * S], F16)   # (e, h*S + q)
        wo_sb = p23.tile([128, HPC * D], F16)
        if with_bias:
            ones512 = consts.tile([1, SB], F16)
            nc.scalar.dma_start(ones512, ones512_d[:])
            ones128 = consts.tile([1, 128], F16)
            nc.scalar.dma_start(ones128, ones128_d[:])
            bq_sb = consts.tile([1, HPC * DH], F16)
            nc.scalar.dma_start(bq_sb, bqf[:])
            bk_sb = consts.tile([1, HPC * DH], F16)
            nc.scalar.dma_start(bk_sb, bkf[:])
            bv_sb = consts.tile([1, HPC * DH], F16)
            nc.scalar.dma_start(bv_sb, bvf[:])

        # ---------------- phase 1: projections + rotary ----------------
        with tc.tile_pool(name="slabs", bufs=4) as slabs, \
             tc.tile_pool(name="rot", bufs=2) as rot:
            # sb0 slabs + remaining weights, all on the sync ring, strictly
            # need-ordered: xq chunks (interleaved with wq heads and the sb0
            # cos/sin columns), then wk, then xk, then the deferred-V inputs.
            xq_s0 = slabs.tile([128, NDT * SB], F16, tag="slab")
            CH = NDT * SB // 4
            nc.sync.dma_start(xq_s0[:, ds(0, CH)], xq[0][:, ds(0, CH)])
            nc.sync.dma_start(cosT[:, 0:SB], cos_d[:, 0:SB])
            nc.sync.dma_start(sinF[:, 0:SB], sin_d[:, 0:SB])
            nc.sync.dma_start(xq_s0[:, ds(1 * CH, CH)], xq[0][:, ds(1 * CH, CH)])
            nc.sync.dma_start(wq_sb[:, ds(1 * QW, QW)], wq[:, ds(1 * QW, QW)])
            nc.sync.dma_start(xq_s0[:, ds(2 * CH, CH)], xq[0][:, ds(2 * CH, CH)])
            nc.sync.dma_start(wq_sb[:, ds(2 * QW, QW)], wq[:, ds(2 * QW, QW)])
            nc.sync.dma_start(xq_s0[:, ds(3 * CH, CH)], xq[0][:, ds(3 * CH, CH)])
            nc.sync.dma_start(wq_sb[:, ds(3 * QW, QW)], wq[:, ds(3 * QW, QW)])
            wk_sb = p1w.tile([128, HPC * NDT * DH], F16)
            xk_s0 = slabs.tile([128, NDT * SB], F16, tag="slab")
            nc.sync.dma_start(wk_sb[:, ds(0, QW)], wk[:, ds(0, QW)])
            nc.sync.dma_start(xk_s0[:, ds(0, CH)], xk[0][:, ds(0, CH)])
            nc.sync.dma_start(wk_sb[:, ds(1 * QW, QW)], wk[:, ds(1 * QW, QW)])
            nc.sync.dma_start(xk_s0[:, ds(1 * CH, CH)], xk[0][:, ds(1 * CH, CH)])
            nc.sync.dma_start(wk_sb[:, ds(2 * QW, QW)], wk[:, ds(2 * QW, QW)])
            nc.sync.dma_start(xk_s0[:, ds(2 * CH, CH)], xk[0][:, ds(2 * CH, CH)])
            nc.sync.dma_start(wk_sb[:, ds(3 * QW, QW)], wk[:, ds(3 * QW, QW)])
            nc.sync.dma_start(xk_s0[:, ds(3 * CH, CH)], xk[0][:, ds(3 * CH, CH)])
            # deferred-V + later-phase inputs, after the startup-critical set
            WVC = NDT * HPC * DH // 4
            for ci in range(4):
                nc.sync.dma_start(wv_sb[:, ds(ci * WVC, WVC)], wv[:, ds(ci * WVC, WVC)])
            xv_s0 = slabs.tile([128, NDT * SB], F16, tag="slab")
            nc.sync.dma_start(xv_s0, xv[0])
            for sbx in range(1, NSB):
                nc.sync.dma_start(cosT[:, ts(sbx, SB)], cos_d[:, ts(sbx, SB)])
                nc.sync.dma_start(sinF[:, ts(sbx, SB)], sin_d[:, ts(sbx, SB)])
            nc.sync.dma_start(mask, mask_d[:])
            nc.sync.dma_start(ones_sq, onesq_d[:])
            # W_O is not needed until ~190us (first out-projection); issuing
            # it here would delay the sb1 slab transfers behind 2MB of dead
            # weight on the FIFO ring, stalling Q(sb1) ~4us. It is issued
            # after sb2's slabs instead (inside the sb loop).

            def rope_evac(pp, dst_slice, sb):
                """dst = rotary(pp) cast to fp16; reads cos/sin columns of block sb.

                The PSUM tile is first evacuated to fp16 SBUF on ACT so the
                rotary multiplies run in the DVE 2x packed mode."""
                q16 = rot.tile([128, SB], F16, tag="q16")
                nc.scalar.copy(q16, pp)
                t1 = rot.tile([128, SB], F16, tag="t1")
                nc.vector.tensor_mul(t1, q16, cosT[:, ts(sb, SB)])
                t2 = rot.tile([128, SB], F16, tag="t2")
                nc.vector.tensor_mul(t2[0:64], q16[64:128], sinF[64:128, ts(sb, SB)])
                nc.vector.tensor_mul(t2[64:128], q16[0:64], sinF[0:64, ts(sb, SB)])
                nc.vector.tensor_add(dst_slice, t1, t2)

            def emit_v(xv_s, sb):
                """V projection for slab sb (deferred one slab iteration so
                xv/wv bytes stay out of the startup DMA-critical window)."""
                for st in range(4):
                    pv = psum.tile([128, HPC * DH], F32, tag="zacc", bufs=2)
                    for dt in range(NDT):
                        nc.tensor.matmul(
                            pv,
                            lhsT=xv_s[:, ds(dt * SB + st * 128, 128)],
                            rhs=wv_sb[:, ts(dt, HPC * DH)],
                            start=(dt == 0), stop=(not with_bias and dt == NDT - 1),
                        )
                    if with_bias:
                        nc.tensor.matmul(pv, lhsT=ones128, rhs=bv_sb,
                                         start=False, stop=True)
                    if sb == NSB - 1:
                        nc.vector.tensor_copy(vsb[:, ts(sb * 4 + st, HPC * DH)], pv)
                    else:
                        nc.scalar.copy(vsb[:, ts(sb * 4 + st, HPC * DH)], pv)

            prev_v = None
            for sb in range(NSB):
                if sb == 0:
                    xq_s, xk_s, xv_s = xq_s0, xk_s0, xv_s0
                else:
                    xq_s = slabs.tile([128, NDT * SB], F16, tag="slab")
                    CH = NDT * SB // 2
                    for ci in range(2):
                        nc.sync.dma_start(
                            xq_s[:, ds(ci * CH, CH)], xq[sb][:, ds(ci * CH, CH)]
                        )
                    xk_s = slabs.tile([128, NDT * SB], F16, tag="slab")
                    nc.sync.dma_start(xk_s, xk[sb])
                    xv_s = slabs.tile([128, NDT * SB], F16, tag="slab")
                    nc.sync.dma_start(xv_s, xv[sb])
                    if sb == 2:
                        nc.sync.dma_start(wo_sb, wo[:])

                # All Q heads first, then all K heads: spreads each slab's
                # full-arrival deadline over ~16us instead of ~4us so the
                # DMA rings can keep up during the startup ramp.
                for h in range(HPC):
                    pq = psum.tile([128, SB], F32, tag="proj", bufs=2)
                    for dt in range(NDT):
                        nc.tensor.matmul(
                            pq,
                            lhsT=wq_sb[:, ds((h * NDT + dt) * DH, DH)],
                            rhs=xq_s[:, ts(dt, SB)],
                            start=(dt == 0), stop=(not with_bias and dt == NDT - 1),
                        )
                    if with_bias:
                        nc.tensor.matmul(pq, lhsT=bq_sb[:, ds(h * DH, DH)],
                                         rhs=ones512, start=False, stop=True)
                    rope_evac(pq, qT[:, ds(h * S + sb * SB, SB)], sb)

                for h in range(HPC):
                    pk = psum.tile([128, SB], F32, tag="scores", bufs=3)
                    for dt in range(NDT):
                        nc.tensor.matmul(
                            pk,
                            lhsT=wk_sb[:, ds((h * NDT + dt) * DH, DH)],
                            rhs=xk_s[:, ts(dt, SB)],
                            start=(dt == 0), stop=(not with_bias and dt == NDT - 1),
                        )
                    if with_bias:
                        nc.tensor.matmul(pk, lhsT=bk_sb[:, ds(h * DH, DH)],
                                         rhs=ones512, start=False, stop=True)
                    rope_evac(pk, kT[:, ds(h * S + sb * SB, SB)], sb)

                if prev_v is not None:
                    emit_v(*prev_v)
                prev_v = (xv_s, sb)
            emit_v(*prev_v)

        # ------- phases 2+3 interleaved: attention + output projection -------
        with tc.tile_pool(name="p2w", bufs=1) as p2w:

            def emit_p3(qb, st_lo=0, st_hi=4):
                for st in range(4 * qb + st_lo, 4 * qb + st_hi):
                    osb = p2w.tile([128, D], F16, tag="osb", bufs=2)
                    for db in range(4):
                        po = psum.tile([128, SB], F32, tag="proj", bufs=2)
                        for h in range(HPC):
                            nc.tensor.matmul(
                                po,
                                lhsT=zT[:, ds(h * S + st * 128, 128)],
                                rhs=wo_sb[:, ds(h * D + db * SB, SB)],
                                start=(h == 0), stop=(h == HPC - 1),
                            )
                        nc.vector.tensor_copy(osb[:, ts(db, SB)], po)
                        nc.sync.dma_start(out_part[ds(st * 128, 128), ts(db, SB)],
                                          osb[:, ts(db, SB)])

            # The per-head finalize (row-sum matmul -> reciprocal -> normalize)
            # is software-pipelined one head behind: the row-sum matmul waits
            # on the DVE pacc-add chain, so issuing it in-line stalls the PE
            # at every head boundary. Deferring it until the next head's kt=1
            # lets its DVE dependencies drain behind independent PE work.
            pending_fin = None

            for qb in range(NSB):
                nkt = 4 * qb + 4
                for h in range(HPC):
                    pz = psum.tile([128, SB], F32, tag="zacc", bufs=2)
                    pacc = p2w.tile([128, SB], F16, tag="pacc", bufs=3)
                    for kt in range(nkt):
                        t = kt - 4 * qb
                        q_lo = 0 if t < 0 else 128 * t
                        pscr = psum.tile([128, SB], F32, tag="scores", bufs=3)
                        nc.tensor.matmul(
                            pscr[:, q_lo:SB],
                            lhsT=kT[:, ds(h * S + kt * 128, 128)],
                            rhs=qT[:, ds(h * S + qb * SB + q_lo, SB - q_lo)],
                            start=True, stop=True,
                        )
                        pt = p2w.tile([128, SB], F16, tag="pt", bufs=8)
                        nc.scalar.activation(
                            pt[:, q_lo:SB], pscr[:, q_lo:SB],
                            mybir.ActivationFunctionType.Exp, scale=SCALE,
                        )
                        if t >= 0:
                            nc.vector.tensor_mul(
                                pt[:, ds(q_lo, 128)], pt[:, ds(q_lo, 128)], mask
                            )
                        nc.tensor.matmul(
                            pz[:, q_lo:SB],
                            lhsT=vsb[:, ds(kt * HPC * DH + h * DH, DH)],
                            rhs=pt[:, q_lo:SB],
                            start=(kt == 0), stop=(kt == nkt - 1),
                        )
                        # batched row-sum accumulation on the DVE
                        if kt == 0:
                            nc.vector.tensor_copy(pacc, pt)
                        else:
                            nc.vector.tensor_add(
                                pacc[:, q_lo:SB], pacc[:, q_lo:SB], pt[:, q_lo:SB]
                            )
                        if kt == 1 and pending_fin is not None:
                            pending_fin()
                            pending_fin = None

                    def _fin(pz=pz, pacc=pacc, h=h, qb=qb):
                        pr = psum.tile([128, SB], F32, tag="rsum", bufs=1)
                        nc.tensor.matmul(pr, lhsT=ones_sq, rhs=pacc,
                                         start=True, stop=True)
                        rr = p2w.tile([128, SB], F32, tag="rr", bufs=2)
                        nc.vector.reciprocal_approx_fast(out=rr, in_=pr)
                        nc.vector.tensor_mul(
                            zT[:, ds(h * S + qb * SB, SB)], pz, rr
                        )
                    pending_fin = _fin

                # Emission lags one q-block so the scheduler can fill exp-
                # latency PE gaps with projection matmuls. qb3's stretch has
                # the most k-tiles, so a slice of qb1's emission is held back
                # to give it extra filler.
                if qb == 1:
                    emit_p3(0)
                elif qb == 2:
                    emit_p3(1, 0, 3)
                elif qb == 3:
                    emit_p3(1, 3, 4)
                    emit_p3(2)
            pending_fin()
            emit_p3(3)

    nc.compile()
    return nc


def _rotary_tables():
    pos = np.arange(S, dtype=np.float64)
    dim = np.arange(DH // 2, dtype=np.float64)
    freq = ROT_BASE ** (dim / (DH / 2))  # base ** (dim / 64)
    freq = np.concatenate([freq, freq])
    angles = pos[:, None] / freq[None, :]          # [S, 128]
    cos_t = np.cos(angles).T.astype(np.float16)    # [128, S]
    sin_t = np.sin(angles).T.astype(np.float16)
    # halves pre-swapped so each rotary mul reads equal base partitions:
    # rows 64:128 = -sin (multiplies q16[64:128] into out[0:64]),
    # rows 0:64   = +sin (multiplies q16[0:64]  into out[64:128])
    sin_f = np.concatenate([sin_t[64:], -sin_t[:64]], axis=0)
    return np.ascontiguousarray(cos_t), np.ascontiguousarray(sin_f)


def _x_slabs(x2d):
    """[S, D] fp32 -> [NSB, 128, NDT*SB] fp16 slab layout of X^T."""
    xt = x2d.T.astype(np.float16)                          # [D, S]
    return np.ascontiguousarray(
        xt.reshape(NDT, 128, NSB, SB).transpose(2, 1, 0, 3).reshape(NSB, 128, NDT * SB)
    )


def _prep_in_maps(inputs, with_bias):
    q_in = np.asarray(inputs["query_input"], np.float32)
    k_in = np.asarray(inputs["key_input"], np.float32)
    v_in = np.asarray(inputs["value_input"], np.float32)
    W_Q = np.asarray(inputs["W_Q"], np.float32)
    W_K = np.asarray(inputs["W_K"], np.float32)
    W_V = np.asarray(inputs["W_V"], np.float32)
    W_O = np.asarray(inputs["W_O"], np.float32)
    b_Q = np.asarray(inputs["b_Q"], np.float32)
    b_K = np.asarray(inputs["b_K"], np.float32)
    b_V = np.asarray(inputs["b_V"], np.float32)

    cos_t, sin_f = _rotary_tables()
    mask_tri = np.triu(np.ones((128, 128), np.float16))    # [k, q]: 1 where k <= q
    ones_sq = np.ones((128, 128), np.float16)

    xq_b = [_x_slabs(q_in[b]) for b in range(B)]
    xk_b = [_x_slabs(k_in[b]) for b in range(B)]
    xv_b = [_x_slabs(v_in[b]) for b in range(B)]

    def w_lhsT(Wg):  # [4, D, DH] -> [128, HPC*NDT*DH]
        return np.ascontiguousarray(
            Wg.reshape(HPC, NDT, 128, DH).transpose(2, 0, 1, 3).reshape(128, -1)
        ).astype(np.float16)

    def w_rhs_v(Wg):  # [4, D, DH] -> [128, NDT*HPC*DH]
        return np.ascontiguousarray(
            Wg.transpose(1, 0, 2).reshape(NDT, 128, HPC * DH)
            .transpose(1, 0, 2).reshape(128, -1)
        ).astype(np.float16)

    def w_rhs_o(Wg):  # [4, DH, D] -> [128, HPC*D]
        return np.ascontiguousarray(Wg.transpose(1, 0, 2).reshape(128, -1)).astype(
            np.float16
        )

    in_maps = []
    for c in range(NCORES):
        b, g = divmod(c, HPC)
        hs = slice(g * HPC, g * HPC + HPC)
        m = {
            "xq": xq_b[b], "xk": xk_b[b], "xv": xv_b[b],
            "wq": w_lhsT(W_Q[hs]), "wk": w_lhsT(W_K[hs]), "wv": w_rhs_v(W_V[hs]),
            "wo": w_rhs_o(W_O[hs]),
            "cos_t": cos_t, "sin_f": sin_f, "mask_tri": mask_tri,
            "ones_sq": ones_sq,
        }
        if with_bias:
            m.update({
                "bqf": b_Q[hs].reshape(1, -1).astype(np.float16),
                "bkf": b_K[hs].reshape(1, -1).astype(np.float16),
                "bvf": b_V[hs].reshape(1, -1).astype(np.float16),
                "ones512": np.ones((1, SB), np.float16),
                "ones128": np.ones((1, 128), np.float16),
            })
        in_maps.append(m)
    return in_maps


_NC_CACHE = {}


def _get_nc(with_bias=False):
    if with_bias not in _NC_CACHE:
        _NC_CACHE[with_bias] = _build_bass(with_bias)
    return _NC_CACHE[with_bias]


def run_sharded(inputs, trace=False, **kwargs):
    """Run the SPMD kernel; returns (full_output, BassKernelResults)."""
    with_bias = any(
        bool(np.any(np.asarray(inputs[k]))) for k in ("b_Q", "b_K", "b_V")
    )
    nc = _get_nc(with_bias)
    in_maps = _prep_in_maps(inputs, with_bias)
    res = run_bass_kernel_spmd(
        nc, in_maps, core_ids=list(range(NCORES)), trace=trace, **kwargs
    )
    b_O = np.asarray(inputs["b_O"], np.float32)
    full = np.zeros((B, S, D), np.float32)
    for c in range(NCORES):
        full[c // HPC] += np.asarray(res.results[c]["out_part"], np.float32)
    full += b_O[None, None, :]
    return full, res


def kernel(**inputs):
    full, _ = run_sharded(inputs, trace=False)
    return full


# revision 60
# speedup vs baseline: 1.1444x; 1.1444x over previous
"""Trainium2 Bass kernel for nn_Attention (B=2, S=2048, D=2048, H=16, DH=128, RoPE, causal).

Sharding: batch (2) x head-groups (4) across 8 cores. Each core computes the
partial output for 1 batch and 4 heads; the host sums the 4 head-group partials
per batch and adds b_O.

Per-core device program (all matmul operands fp16, fp32 PSUM accumulation):
  phase 1: QKV projections from host-pre-transposed X^T slabs; rotary fused into
           the PSUM->SBUF evacuation of Q^T/K^T. All inputs stream on ONE sync
           DMA ring in exact first-need order (Q heads, then K heads, then the
           one-slab-deferred V projection) so prefetches never steal SDMA
           bandwidth from the startup-critical stream.
  phase 2 (q-blocks ascending, interleaved with phase 3 by the Tile scheduler):
           per (head, q-block of 512): scores^T tiles = K_tile^T.T @ Q^T (causal
           block-skipping), exp on ACT with 1/sqrt(128) folded into the scale,
           triangular mask-mul on the diagonal 128x128 sub-block, AV accumulated
           over k-tiles. Row sums are batched: exp tiles are accumulated on the
           DVE into one fp16 tile per (head, q-block) and a SINGLE all-ones
           matmul produces the partition-replicated denominators (vs one such
           matmul per k-tile) -- saves ~9% of PE column-cycles. Fast approx
           reciprocal (custom DVE op), single normalize multiply.
  phase 3: output projection out[s,d] += Z^T[h].T @ W_O[h], fp16 partials DMAd
           out; emitted one q-block behind phase 2 so the scheduler can fill
           exp-latency PE gaps with projection matmuls.
"""

import os
import sys

if "/opt/trn_rl_repo" not in sys.path:
    sys.path.insert(0, "/opt/trn_rl_repo")

from contextlib import ExitStack

import numpy as np

import concourse.bass as bass
import concourse.tile as tile
from concourse import bacc, mybir
from concourse.bass import ds, ts
from concourse.bass_utils import run_bass_kernel_spmd

B, S, D, H, DH = 2, 2048, 2048, 16, 128
HPC = 4            # heads per core
NCORES = 8
SB = 512           # s/q block width
NSB = S // SB      # 4
NDT = D // 128     # 16 contraction d-tiles
NST = S // 128     # 16 s-tiles / k-tiles
ROT_BASE = 10000.0
SCALE = 1.0 / float(np.sqrt(float(DH)))

F16 = mybir.dt.float16
F32 = mybir.dt.float32


def _build_bass(with_bias):
    nc = bacc.Bacc()

    # --- I/O ---
    xq = nc.dram_tensor("xq", [NSB, 128, NDT * SB], F16, kind="ExternalInput")
    xk = nc.dram_tensor("xk", [NSB, 128, NDT * SB], F16, kind="ExternalInput")
    xv = nc.dram_tensor("xv", [NSB, 128, NDT * SB], F16, kind="ExternalInput")
    wq = nc.dram_tensor("wq", [128, HPC * NDT * DH], F16, kind="ExternalInput")
    wk = nc.dram_tensor("wk", [128, HPC * NDT * DH], F16, kind="ExternalInput")
    wv = nc.dram_tensor("wv", [128, NDT * HPC * DH], F16, kind="ExternalInput")
    wo = nc.dram_tensor("wo", [128, HPC * D], F16, kind="ExternalInput")
    if with_bias:
        bqf = nc.dram_tensor("bqf", [1, HPC * DH], F16, kind="ExternalInput")
        bkf = nc.dram_tensor("bkf", [1, HPC * DH], F16, kind="ExternalInput")
        bvf = nc.dram_tensor("bvf", [1, HPC * DH], F16, kind="ExternalInput")
        ones512_d = nc.dram_tensor("ones512", [1, SB], F16, kind="ExternalInput")
        ones128_d = nc.dram_tensor("ones128", [1, 128], F16, kind="ExternalInput")
    cos_d = nc.dram_tensor("cos_t", [128, S], F16, kind="ExternalInput")
    sin_d = nc.dram_tensor("sin_f", [128, S], F16, kind="ExternalInput")
    mask_d = nc.dram_tensor("mask_tri", [128, 128], F16, kind="ExternalInput")
    onesq_d = nc.dram_tensor("ones_sq", [128, 128], F16, kind="ExternalInput")
    out_part = nc.dram_tensor("out_part", [S, D], F16, kind="ExternalOutput")

    with ExitStack() as ctx:
        tc = ctx.enter_context(tile.TileContext(nc))

        glob = ctx.enter_context(tc.tile_pool(name="glob", bufs=1))
        psum = ctx.enter_context(tc.tile_pool(name="psum", bufs=1, space="PSUM"))
        persist = consts = p1w = p23 = glob

        # persistent activations
        qT = persist.tile([128, HPC * S], F16)   # (e, h*S + s)
        kT = persist.tile([128, HPC * S], F16)   # (e, h*S + s)
        vsb = persist.tile([128, NST * HPC * DH], F16)  # (s%128, stile*512 + h*128 + e)

        # constants
        mask = consts.tile([128, 128], F16)
        ones_sq = consts.tile([128, 128], F16)

        warm_sb = consts.tile([128, SB], F16)
        nc.vector.memset(warm_sb, 1.0)

        QW = NDT * DH  # per-head weight columns

        # ALL phase-1 input DMAs go on the ONE sync ring in exact first-need
        # order: a single HWDGE ring saturates HBM by itself, and FIFO order
        # means later (prefetch) transfers can never steal SDMA bandwidth
        # from the critical startup stream.
        wq_sb = p1w.tile([128, HPC * NDT * DH], F16)
        cosT = p1w.tile([128, S], F16)
        sinF = p1w.tile([128, S], F16)
        wv_sb = p1w.tile([128, NDT * HPC * DH], F16)

        # Preload the ACT exp table while the startup is DMA-bound (the
        # ACT_TABLE_LOAD walrus inserts before the first Exp costs ~1.5us).
        exp_warm = consts.tile([1, 16], F16)
        nc.scalar.activation(exp_warm, warm_sb[0:1, 0:16],
                             mybir.ActivationFunctionType.Exp, scale=SCALE)

        # HAM warm-up: keep the PE busy during the DMA-bound startup so the
        # first real matmuls run un-throttled (results never read).
        warm_ps = psum.tile([128, SB], F32, tag="rsum", bufs=1)
        for _ in range(13):
            nc.tensor.matmul(warm_ps, lhsT=warm_sb[:, 0:128], rhs=warm_sb,
                             start=True, stop=True)

        # phase-3 persistents (W_O DMA issued at the end of the need-ordered
        # sync chain, inside the slab block below).
        zT = p23.tile([128, HPC * S], F16)   # (e, h*S + q)
        wo_sb = p23.tile([128, HPC * D], F16)
        if with_bias:
            ones512 = consts.tile([1, SB], F16)
            nc.scalar.dma_start(ones512, ones512_d[:])
            ones128 = consts.tile([1, 128], F16)
            nc.scalar.dma_start(ones128, ones128_d[:])
            bq_sb = consts.tile([1, HPC * DH], F16)
            nc.scalar.dma_start(bq_sb, bqf[:])
            bk_sb = consts.tile([1, HPC * DH], F16)
            nc.scalar.dma_start(bk_sb, bkf[:])
            bv_sb = consts.tile([1, HPC * DH], F16)
            nc.scalar.dma_start(bv_sb, bvf[:])

        # ---------------- phase 1: projections + rotary ----------------
        with tc.tile_pool(name="slabs", bufs=4) as slabs, \
             tc.tile_pool(name="rot", bufs=2) as rot:
            # sb0 slabs + remaining weights, all on the sync ring, strictly
            # need-ordered: xq chunks (interleaved with wq heads and the sb0
            # cos/sin columns), then wk, then xk, then the deferred-V inputs.
            xq_s0 = slabs.tile([128, NDT * SB], F16, tag="slab")
            CH = NDT * SB // 4
            HQ = QW // 2
            nc.sync.dma_start(wq_sb[:, ds(0, HQ)], wq[:, ds(0, HQ)])
            nc.sync.dma_start(xq_s0[:, ds(0, CH // 2)], xq[0][:, ds(0, CH // 2)])
            nc.sync.dma_start(wq_sb[:, ds(HQ, HQ)], wq[:, ds(HQ, HQ)])
            nc.sync.dma_start(xq_s0[:, ds(CH // 2, CH // 2)],
                              xq[0][:, ds(CH // 2, CH // 2)])
            nc.sync.dma_start(cosT[:, 0:SB], cos_d[:, 0:SB])
            nc.sync.dma_start(sinF[:, 0:SB], sin_d[:, 0:SB])
            nc.sync.dma_start(xq_s0[:, ds(1 * CH, CH)], xq[0][:, ds(1 * CH, CH)])
            nc.sync.dma_start(wq_sb[:, ds(1 * QW, QW)], wq[:, ds(1 * QW, QW)])
            nc.sync.dma_start(xq_s0[:, ds(2 * CH, CH)], xq[0][:, ds(2 * CH, CH)])
            nc.sync.dma_start(wq_sb[:, ds(2 * QW, QW)], wq[:, ds(2 * QW, QW)])
            nc.sync.dma_start(xq_s0[:, ds(3 * CH, CH)], xq[0][:, ds(3 * CH, CH)])
            nc.sync.dma_start(wq_sb[:, ds(3 * QW, QW)], wq[:, ds(3 * QW, QW)])
            wk_sb = p1w.tile([128, HPC * NDT * DH], F16)
            xk_s0 = slabs.tile([128, NDT * SB], F16, tag="slab")
            nc.sync.dma_start(wk_sb[:, ds(0, QW)], wk[:, ds(0, QW)])
            nc.sync.dma_start(xk_s0[:, ds(0, CH)], xk[0][:, ds(0, CH)])
            nc.sync.dma_start(wk_sb[:, ds(1 * QW, QW)], wk[:, ds(1 * QW, QW)])
            nc.sync.dma_start(xk_s0[:, ds(1 * CH, CH)], xk[0][:, ds(1 * CH, CH)])
            nc.sync.dma_start(wk_sb[:, ds(2 * QW, QW)], wk[:, ds(2 * QW, QW)])
            nc.sync.dma_start(xk_s0[:, ds(2 * CH, CH)], xk[0][:, ds(2 * CH, CH)])
            nc.sync.dma_start(wk_sb[:, ds(3 * QW, QW)], wk[:, ds(3 * QW, QW)])
            nc.sync.dma_start(xk_s0[:, ds(3 * CH, CH)], xk[0][:, ds(3 * CH, CH)])
            # deferred-V + later-phase inputs, after the startup-critical set
            WVC = NDT * HPC * DH // 4
            for ci in range(4):
                nc.sync.dma_start(wv_sb[:, ds(ci * WVC, WVC)], wv[:, ds(ci * WVC, WVC)])
            xv_s0 = slabs.tile([128, NDT * SB], F16, tag="slab")
            nc.sync.dma_start(xv_s0, xv[0])
            for sbx in range(1, NSB):
                nc.sync.dma_start(cosT[:, ts(sbx, SB)], cos_d[:, ts(sbx, SB)])
                nc.sync.dma_start(sinF[:, ts(sbx, SB)], sin_d[:, ts(sbx, SB)])
            nc.sync.dma_start(mask, mask_d[:])
            nc.sync.dma_start(ones_sq, onesq_d[:])
            # W_O is not needed until ~190us (first out-projection); issuing
            # it here would delay the sb1 slab transfers behind 2MB of dead
            # weight on the FIFO ring, stalling Q(sb1) ~4us. It is issued
            # after sb2's slabs instead (inside the sb loop).

            def rope_evac(pp, dst_slice, sb):
                """dst = rotary(pp) cast to fp16; reads cos/sin columns of block sb.

                The PSUM tile is first evacuated to fp16 SBUF on ACT so the
                rotary multiplies run in the DVE 2x packed mode."""
                q16 = rot.tile([128, SB], F16, tag="q16")
                nc.scalar.copy(q16, pp)
                t1 = rot.tile([128, SB], F16, tag="t1")
                nc.vector.tensor_mul(t1, q16, cosT[:, ts(sb, SB)])
                t2 = rot.tile([128, SB], F16, tag="t2")
                nc.vector.tensor_mul(t2[0:64], q16[64:128], sinF[64:128, ts(sb, SB)])
                nc.vector.tensor_mul(t2[64:128], q16[0:64], sinF[0:64, ts(sb, SB)])
                nc.vector.tensor_add(dst_slice, t1, t2)

            def emit_v(xv_s, sb):
                """V projection for slab sb (deferred one slab iteration so
                xv/wv bytes stay out of the startup DMA-critical window)."""
                for st in range(4):
                    pv = psum.tile([128, HPC * DH], F32, tag="zacc", bufs=2)
                    for dt in range(NDT):
                        nc.tensor.matmul(
                            pv,
                            lhsT=xv_s[:, ds(dt * SB + st * 128, 128)],
                            rhs=wv_sb[:, ts(dt, HPC * DH)],
                            start=(dt == 0), stop=(not with_bias and dt == NDT - 1),
                        )
                    if with_bias:
                        nc.tensor.matmul(pv, lhsT=ones128, rhs=bv_sb,
                                         start=False, stop=True)
                    if sb == NSB - 1:
                        nc.vector.tensor_copy(vsb[:, ts(sb * 4 + st, HPC * DH)], pv)
                    else:
                        nc.scalar.copy(vsb[:, ts(sb * 4 + st, HPC * DH)], pv)

            prev_v = None
            for sb in range(NSB):
                if sb == 0:
                    xq_s, xk_s, xv_s = xq_s0, xk_s0, xv_s0
                else:
                    xq_s = slabs.tile([128, NDT * SB], F16, tag="slab")
                    CH = NDT * SB // 2
                    for ci in range(2):
                        nc.sync.dma_start(
                            xq_s[:, ds(ci * CH, CH)], xq[sb][:, ds(ci * CH, CH)]
                        )
                    xk_s = slabs.tile([128, NDT * SB], F16, tag="slab")
                    nc.sync.dma_start(xk_s, xk[sb])
                    xv_s = slabs.tile([128, NDT * SB], F16, tag="slab")
                    nc.sync.dma_start(xv_s, xv[sb])
                    if sb == 2:
                        nc.sync.dma_start(wo_sb, wo[:])

                # All Q heads first, then all K heads: spreads each slab's
                # full-arrival deadline over ~16us instead of ~4us so the
                # DMA rings can keep up during the startup ramp.
                for h in range(HPC):
                    pq = psum.tile([128, SB], F32, tag="proj", bufs=2)
                    for dt in range(NDT):
                        nc.tensor.matmul(
                            pq,
                            lhsT=wq_sb[:, ds((h * NDT + dt) * DH, DH)],
                            rhs=xq_s[:, ts(dt, SB)],
                            start=(dt == 0), stop=(not with_bias and dt == NDT - 1),
                        )
                    if with_bias:
                        nc.tensor.matmul(pq, lhsT=bq_sb[:, ds(h * DH, DH)],
                                         rhs=ones512, start=False, stop=True)
                    rope_evac(pq, qT[:, ds(h * S + sb * SB, SB)], sb)

                for h in range(HPC):
                    pk = psum.tile([128, SB], F32, tag="scores", bufs=3)
                    for dt in range(NDT):
                        nc.tensor.matmul(
                            pk,
                            lhsT=wk_sb[:, ds((h * NDT + dt) * DH, DH)],
                            rhs=xk_s[:, ts(dt, SB)],
                            start=(dt == 0), stop=(not with_bias and dt == NDT - 1),
                        )
                    if with_bias:
                        nc.tensor.matmul(pk, lhsT=bk_sb[:, ds(h * DH, DH)],
                                         rhs=ones512, start=False, stop=True)
                    rope_evac(pk, kT[:, ds(h * S + sb * SB, SB)], sb)

                if prev_v is not None:
                    emit_v(*prev_v)
                prev_v = (xv_s, sb)
            emit_v(*prev_v)

        # ------- phases 2+3 interleaved: attention + output projection -------
        with tc.tile_pool(name="p2w", bufs=1) as p2w:

            def emit_p3(qb, st_lo=0, st_hi=4):
                for st in range(4 * qb + st_lo, 4 * qb + st_hi):
                    osb = p2w.tile([128, D], F16, tag="osb", bufs=2)
                    for db in range(4):
                        po = psum.tile([128, SB], F32, tag="proj", bufs=2)
                        for h in range(HPC):
                            nc.tensor.matmul(
                                po,
                                lhsT=zT[:, ds(h * S + st * 128, 128)],
                                rhs=wo_sb[:, ds(h * D + db * SB, SB)],
                                start=(h == 0), stop=(h == HPC - 1),
                            )
                        nc.vector.tensor_copy(osb[:, ts(db, SB)], po)
                        nc.sync.dma_start(out_part[ds(st * 128, 128), ts(db, SB)],
                                          osb[:, ts(db, SB)])

            # The per-head finalize (row-sum matmul -> reciprocal -> normalize)
            # is software-pipelined one head behind: the row-sum matmul waits
            # on the DVE pacc-add chain, so issuing it in-line stalls the PE
            # at every head boundary. Deferring it until the next head's kt=1
            # lets its DVE dependencies drain behind independent PE work.
            pending_fin = None

            for qb in range(NSB):
                nkt = 4 * qb + 4
                for h in range(HPC):
                    pz = psum.tile([128, SB], F32, tag="zacc", bufs=2)
                    pacc = p2w.tile([128, SB], F16, tag="pacc", bufs=3)
                    for kt in range(nkt):
                        t = kt - 4 * qb
                        q_lo = 0 if t < 0 else 128 * t
                        pscr = psum.tile([128, SB], F32, tag="scores", bufs=3)
                        nc.tensor.matmul(
                            pscr[:, q_lo:SB],
                            lhsT=kT[:, ds(h * S + kt * 128, 128)],
                            rhs=qT[:, ds(h * S + qb * SB + q_lo, SB - q_lo)],
                            start=True, stop=True,
                        )
                        pt = p2w.tile([128, SB], F16, tag="pt", bufs=8)
                        nc.scalar.activation(
                            pt[:, q_lo:SB], pscr[:, q_lo:SB],
                            mybir.ActivationFunctionType.Exp, scale=SCALE,
                        )
                        if t >= 0:
                            nc.vector.tensor_mul(
                                pt[:, ds(q_lo, 128)], pt[:, ds(q_lo, 128)], mask
                            )
                        nc.tensor.matmul(
                            pz[:, q_lo:SB],
                            lhsT=vsb[:, ds(kt * HPC * DH + h * DH, DH)],
                            rhs=pt[:, q_lo:SB],
                            start=(kt == 0), stop=(kt == nkt - 1),
                        )
                        # batched row-sum accumulation on the DVE
                        if kt == 0:
                            nc.vector.tensor_copy(pacc, pt)
                        else:
                            nc.vector.tensor_add(
                                pacc[:, q_lo:SB], pacc[:, q_lo:SB], pt[:, q_lo:SB]
                            )
                        if kt == 1 and pending_fin is not None:
                            pending_fin()
                            pending_fin = None

                    def _fin(pz=pz, pacc=pacc, h=h, qb=qb):
                        pr = psum.tile([128, SB], F32, tag="rsum", bufs=1)
                        nc.tensor.matmul(pr, lhsT=ones_sq, rhs=pacc,
                                         start=True, stop=True)
                        rr = p2w.tile([128, SB], F32, tag="rr", bufs=2)
                        nc.vector.reciprocal_approx_fast(out=rr, in_=pr)
                        nc.vector.tensor_mul(
                            zT[:, ds(h * S + qb * SB, SB)], pz, rr
                        )
                    pending_fin = _fin

                # Emission lags one q-block so the scheduler can fill exp-
                # latency PE gaps with projection matmuls. qb3's stretch has
                # the most k-tiles, so a slice of qb1's emission is held back
                # to give it extra filler.
                if qb == 1:
                    emit_p3(0)
                elif qb == 2:
                    emit_p3(1, 0, 3)
                elif qb == 3:
                    emit_p3(1, 3, 4)
                    emit_p3(2)
            pending_fin()
            emit_p3(3)

    nc.compile()
    return nc


def _rotary_tables():
    pos = np.arange(S, dtype=np.float64)
    dim = np.arange(DH // 2, dtype=np.float64)
    freq = ROT_BASE ** (dim / (DH / 2))  # base ** (dim / 64)
    freq = np.concatenate([freq, freq])
    angles = pos[:, None] / freq[None, :]          # [S, 128]
    cos_t = np.cos(angles).T.astype(np.float16)    # [128, S]
    sin_t = np.sin(angles).T.astype(np.float16)
    # halves pre-swapped so each rotary mul reads equal base partitions:
    # rows 64:128 = -sin (multiplies q16[64:128] into out[0:64]),
    # rows 0:64   = +sin (multiplies q16[0:64]  into out[64:128])
    sin_f = np.concatenate([sin_t[64:], -sin_t[:64]], axis=0)
    return np.ascontiguousarray(cos_t), np.ascontiguousarray(sin_f)


def _x_slabs(x2d):
    """[S, D] fp32 -> [NSB, 128, NDT*SB] fp16 slab layout of X^T."""
    xt = x2d.T.astype(np.float16)                          # [D, S]
    return np.ascontiguousarray(
        xt.reshape(NDT, 128, NSB, SB).transpose(2, 1, 0, 3).reshape(NSB, 128, NDT * SB)
    )


def _prep_in_maps(inputs, with_bias):
    q_in = np.asarray(inputs["query_input"], np.float32)
    k_in = np.asarray(inputs["key_input"], np.float32)
    v_in = np.asarray(inputs["value_input"], np.float32)
    W_Q = np.asarray(inputs["W_Q"], np.float32)
    W_K = np.asarray(inputs["W_K"], np.float32)
    W_V = np.asarray(inputs["W_V"], np.float32)
    W_O = np.asarray(inputs["W_O"], np.float32)
    b_Q = np.asarray(inputs["b_Q"], np.float32)
    b_K = np.asarray(inputs["b_K"], np.float32)
    b_V = np.asarray(inputs["b_V"], np.float32)

    cos_t, sin_f = _rotary_tables()
    mask_tri = np.triu(np.ones((128, 128), np.float16))    # [k, q]: 1 where k <= q
    ones_sq = np.ones((128, 128), np.float16)

    xq_b = [_x_slabs(q_in[b]) for b in range(B)]
    xk_b = [_x_slabs(k_in[b]) for b in range(B)]
    xv_b = [_x_slabs(v_in[b]) for b in range(B)]

    def w_lhsT(Wg):  # [4, D, DH] -> [128, HPC*NDT*DH]
        return np.ascontiguousarray(
            Wg.reshape(HPC, NDT, 128, DH).transpose(2, 0, 1, 3).reshape(128, -1)
        ).astype(np.float16)

    def w_rhs_v(Wg):  # [4, D, DH] -> [128, NDT*HPC*DH]
        return np.ascontiguousarray(
            Wg.transpose(1, 0, 2).reshape(NDT, 128, HPC * DH)
            .transpose(1, 0, 2).reshape(128, -1)
        ).astype(np.float16)

    def w_rhs_o(Wg):  # [4, DH, D] -> [128, HPC*D]
        return np.ascontiguousarray(Wg.transpose(1, 0, 2).reshape(128, -1)).astype(
            np.float16
        )

    in_maps = []
    for c in range(NCORES):
        b, g = divmod(c, HPC)
        hs = slice(g * HPC, g * HPC + HPC)
        m = {
            "xq": xq_b[b], "xk": xk_b[b], "xv": xv_b[b],
            "wq": w_lhsT(W_Q[hs]), "wk": w_lhsT(W_K[hs]), "wv": w_rhs_v(W_V[hs]),
            "wo": w_rhs_o(W_O[hs]),
            "cos_t": cos_t, "sin_f": sin_f, "mask_tri": mask_tri,
            "ones_sq": ones_sq,
        }
        if with_bias:
            m.update({
                "bqf": b_Q[hs].reshape(1, -1).astype(np.float16),
                "bkf": b_K[hs].reshape(1, -1).astype(np.float16),
                "bvf": b_V[hs].reshape(1, -1).astype(np.float16),
                "ones512": np.ones((1, SB), np.float16),
                "ones128": np.ones((1, 128), np.float16),
            })
        in_maps.append(m)
    return in_maps


_NC_CACHE = {}


def _get_nc(with_bias=False):
    if with_bias not in _NC_CACHE:
        _NC_CACHE[with_bias] = _build_bass(with_bias)
    return _NC_CACHE[with_bias]


def run_sharded(inputs, trace=False, **kwargs):
    """Run the SPMD kernel; returns (full_output, BassKernelResults)."""
    with_bias = any(
        bool(np.any(np.asarray(inputs[k]))) for k in ("b_Q", "b_K", "b_V")
    )
    nc = _get_nc(with_bias)
    in_maps = _prep_in_maps(inputs, with_bias)
    res = run_bass_kernel_spmd(
        nc, in_maps, core_ids=list(range(NCORES)), trace=trace, **kwargs
    )
    b_O = np.asarray(inputs["b_O"], np.float32)
    full = np.zeros((B, S, D), np.float32)
    for c in range(NCORES):
        full[c // HPC] += np.asarray(res.results[c]["out_part"], np.float32)
    full += b_O[None, None, :]
    return full, res


def kernel(**inputs):
    full, _ = run_sharded(inputs, trace=False)
    return full


# revision 61
# speedup vs baseline: 1.1927x; 1.0423x over previous
"""Trainium2 Bass kernel for nn_Attention (B=2, S=2048, D=2048, H=16, DH=128, RoPE, causal).

Sharding: batch (2) x head-groups (4) across 8 cores. Each core computes the
partial output for 1 batch and 4 heads; the host sums the 4 head-group partials
per batch and adds b_O.

Per-core device program (all matmul operands fp16, fp32 PSUM accumulation):
  phase 1: QKV projections from host-pre-transposed X^T slabs; rotary fused into
           the PSUM->SBUF evacuation of Q^T/K^T. All inputs stream on ONE sync
           DMA ring in exact first-need order (Q heads, then K heads, then the
           one-slab-deferred V projection) so prefetches never steal SDMA
           bandwidth from the startup-critical stream.
  phase 2 (q-blocks ascending, interleaved with phase 3 by the Tile scheduler):
           per (head, q-block of 512): scores^T tiles = K_tile^T.T @ Q^T (causal
           block-skipping), exp on ACT with 1/sqrt(128) folded into the scale,
           triangular mask-mul on the diagonal 128x128 sub-block, AV accumulated
           over k-tiles. Row sums are batched: exp tiles are accumulated on the
           DVE into one fp16 tile per (head, q-block) and a SINGLE all-ones
           matmul produces the partition-replicated denominators (vs one such
           matmul per k-tile) -- saves ~9% of PE column-cycles. Fast approx
           reciprocal (custom DVE op), single normalize multiply.
  phase 3: output projection out[s,d] += Z^T[h].T @ W_O[h], fp16 partials DMAd
           out; emitted one q-block behind phase 2 so the scheduler can fill
           exp-latency PE gaps with projection matmuls.
"""

import os
import sys

if "/opt/trn_rl_repo" not in sys.path:
    sys.path.insert(0, "/opt/trn_rl_repo")

from contextlib import ExitStack

import numpy as np

import concourse.bass as bass
import concourse.tile as tile
from concourse import bacc, mybir
from concourse.bass import ds, ts
from concourse.bass_utils import run_bass_kernel_spmd

B, S, D, H, DH = 2, 2048, 2048, 16, 128
HPC = 4            # heads per core
NCORES = 8
SB = 512           # s/q block width
NSB = S // SB      # 4
NDT = D // 128     # 16 contraction d-tiles
NST = S // 128     # 16 s-tiles / k-tiles
ROT_BASE = 10000.0
SCALE = 1.0 / float(np.sqrt(float(DH)))

F16 = mybir.dt.float16
F32 = mybir.dt.float32


def _build_bass(with_bias):
    nc = bacc.Bacc()

    # --- I/O ---
    xq = nc.dram_tensor("xq", [NSB, 128, NDT * SB], F16, kind="ExternalInput")
    xk = nc.dram_tensor("xk", [NSB, 128, NDT * SB], F16, kind="ExternalInput")
    xv = nc.dram_tensor("xv", [NSB, 128, NDT * SB], F16, kind="ExternalInput")
    wq = nc.dram_tensor("wq", [128, HPC * NDT * DH], F16, kind="ExternalInput")
    wk = nc.dram_tensor("wk", [128, HPC * NDT * DH], F16, kind="ExternalInput")
    wv = nc.dram_tensor("wv", [128, NDT * HPC * DH], F16, kind="ExternalInput")
    wo = nc.dram_tensor("wo", [128, HPC * D], F16, kind="ExternalInput")
    if with_bias:
        bqf = nc.dram_tensor("bqf", [1, HPC * DH], F16, kind="ExternalInput")
        bkf = nc.dram_tensor("bkf", [1, HPC * DH], F16, kind="ExternalInput")
        bvf = nc.dram_tensor("bvf", [1, HPC * DH], F16, kind="ExternalInput")
        ones512_d = nc.dram_tensor("ones512", [1, SB], F16, kind="ExternalInput")
        ones128_d = nc.dram_tensor("ones128", [1, 128], F16, kind="ExternalInput")
    cos_d = nc.dram_tensor("cos_t", [128, S], F16, kind="ExternalInput")
    sin_d = nc.dram_tensor("sin_f", [128, S], F16, kind="ExternalInput")
    mask_d = nc.dram_tensor("mask_tri", [128, 128], F16, kind="ExternalInput")
    onesq_d = nc.dram_tensor("ones_sq", [128, 128], F16, kind="ExternalInput")
    out_part = nc.dram_tensor("out_part", [S, D], F16, kind="ExternalOutput")

    with ExitStack() as ctx:
        tc = ctx.enter_context(tile.TileContext(nc))

        glob = ctx.enter_context(tc.tile_pool(name="glob", bufs=1))
        psum = ctx.enter_context(tc.tile_pool(name="psum", bufs=1, space="PSUM"))
        persist = consts = p1w = p23 = glob

        # persistent activations
        qT = persist.tile([128, HPC * S], F16)   # (e, h*S + s)
        kT = persist.tile([128, HPC * S], F16)   # (e, h*S + s)
        vsb = persist.tile([128, NST * HPC * DH], F16)  # (s%128, stile*512 + h*128 + e)

        # constants
        mask = consts.tile([128, 128], F16)
        ones_sq = consts.tile([128, 128], F16)

        warm_sb = consts.tile([128, SB], F16)
        nc.vector.memset(warm_sb, 1.0)

        QW = NDT * DH  # per-head weight columns

        # ALL phase-1 input DMAs go on the ONE sync ring in exact first-need
        # order: a single HWDGE ring saturates HBM by itself, and FIFO order
        # means later (prefetch) transfers can never steal SDMA bandwidth
        # from the critical startup stream.
        wq_sb = p1w.tile([128, HPC * NDT * DH], F16)
        nc.sync.dma_start(wq_sb[:, 0:QW], wq[:, 0:QW])
        cosT = p1w.tile([128, S], F16)
        sinF = p1w.tile([128, S], F16)
        wv_sb = p1w.tile([128, NDT * HPC * DH], F16)

        # Preload the ACT exp table while the startup is DMA-bound (the
        # ACT_TABLE_LOAD walrus inserts before the first Exp costs ~1.5us).
        exp_warm = consts.tile([1, 16], F16)
        nc.scalar.activation(exp_warm, warm_sb[0:1, 0:16],
                             mybir.ActivationFunctionType.Exp, scale=SCALE)

        # HAM warm-up: keep the PE busy during the DMA-bound startup so the
        # first real matmuls run un-throttled (results never read).
        warm_ps = psum.tile([128, SB], F32, tag="rsum", bufs=1)
        for _ in range(21):
            nc.tensor.matmul(warm_ps, lhsT=warm_sb[:, 0:128], rhs=warm_sb,
                             start=True, stop=True)

        # phase-3 persistents (W_O DMA issued at the end of the need-ordered
        # sync chain, inside the slab block below).
        zT = p23.tile([128, HPC * S], F16)   # (e, h*S + q)
        wo_sb = p23.tile([128, HPC * D], F16)
        if with_bias:
            ones512 = consts.tile([1, SB], F16)
            nc.scalar.dma_start(ones512, ones512_d[:])
            ones128 = consts.tile([1, 128], F16)
            nc.scalar.dma_start(ones128, ones128_d[:])
            bq_sb = consts.tile([1, HPC * DH], F16)
            nc.scalar.dma_start(bq_sb, bqf[:])
            bk_sb = consts.tile([1, HPC * DH], F16)
            nc.scalar.dma_start(bk_sb, bkf[:])
            bv_sb = consts.tile([1, HPC * DH], F16)
            nc.scalar.dma_start(bv_sb, bvf[:])

        # ---------------- phase 1: projections + rotary ----------------
        with tc.tile_pool(name="slabs", bufs=4) as slabs, \
             tc.tile_pool(name="rot", bufs=2) as rot:
            # sb0 slabs + remaining weights, all on the sync ring, strictly
            # need-ordered: xq chunks (interleaved with wq heads and the sb0
            # cos/sin columns), then wk, then xk, then the deferred-V inputs.
            xq_s0 = slabs.tile([128, NDT * SB], F16, tag="slab")
            CH = NDT * SB // 4
            nc.sync.dma_start(xq_s0[:, ds(0, CH)], xq[0][:, ds(0, CH)])
            nc.sync.dma_start(cosT[:, 0:SB], cos_d[:, 0:SB])
            nc.sync.dma_start(sinF[:, 0:SB], sin_d[:, 0:SB])
            nc.sync.dma_start(xq_s0[:, ds(1 * CH, CH)], xq[0][:, ds(1 * CH, CH)])
            nc.sync.dma_start(wq_sb[:, ds(1 * QW, QW)], wq[:, ds(1 * QW, QW)])
            nc.sync.dma_start(xq_s0[:, ds(2 * CH, CH)], xq[0][:, ds(2 * CH, CH)])
            nc.sync.dma_start(wq_sb[:, ds(2 * QW, QW)], wq[:, ds(2 * QW, QW)])
            nc.sync.dma_start(xq_s0[:, ds(3 * CH, CH)], xq[0][:, ds(3 * CH, CH)])
            nc.sync.dma_start(wq_sb[:, ds(3 * QW, QW)], wq[:, ds(3 * QW, QW)])
            wk_sb = p1w.tile([128, HPC * NDT * DH], F16)
            xk_s0 = slabs.tile([128, NDT * SB], F16, tag="slab")
            nc.sync.dma_start(wk_sb[:, ds(0, QW)], wk[:, ds(0, QW)])
            nc.sync.dma_start(xk_s0[:, ds(0, CH)], xk[0][:, ds(0, CH)])
            nc.sync.dma_start(wk_sb[:, ds(1 * QW, QW)], wk[:, ds(1 * QW, QW)])
            nc.sync.dma_start(xk_s0[:, ds(1 * CH, CH)], xk[0][:, ds(1 * CH, CH)])
            nc.sync.dma_start(wk_sb[:, ds(2 * QW, QW)], wk[:, ds(2 * QW, QW)])
            nc.sync.dma_start(xk_s0[:, ds(2 * CH, CH)], xk[0][:, ds(2 * CH, CH)])
            nc.sync.dma_start(wk_sb[:, ds(3 * QW, QW)], wk[:, ds(3 * QW, QW)])
            nc.sync.dma_start(xk_s0[:, ds(3 * CH, CH)], xk[0][:, ds(3 * CH, CH)])
            # deferred-V + later-phase inputs, after the startup-critical set
            WVC = NDT * HPC * DH // 4
            for ci in range(4):
                nc.sync.dma_start(wv_sb[:, ds(ci * WVC, WVC)], wv[:, ds(ci * WVC, WVC)])
            xv_s0 = slabs.tile([128, NDT * SB], F16, tag="slab")
            nc.sync.dma_start(xv_s0, xv[0])
            for sbx in range(1, NSB):
                nc.sync.dma_start(cosT[:, ts(sbx, SB)], cos_d[:, ts(sbx, SB)])
                nc.sync.dma_start(sinF[:, ts(sbx, SB)], sin_d[:, ts(sbx, SB)])
            nc.sync.dma_start(mask, mask_d[:])
            nc.sync.dma_start(ones_sq, onesq_d[:])
            # W_O is not needed until ~190us (first out-projection); issuing
            # it here would delay the sb1 slab transfers behind 2MB of dead
            # weight on the FIFO ring, stalling Q(sb1) ~4us. It is issued
            # after sb2's slabs instead (inside the sb loop).

            def rope_evac(pp, dst_slice, sb):
                """dst = rotary(pp) cast to fp16; reads cos/sin columns of block sb.

                The PSUM tile is first evacuated to fp16 SBUF on ACT so the
                rotary multiplies run in the DVE 2x packed mode."""
                q16 = rot.tile([128, SB], F16, tag="q16")
                nc.scalar.copy(q16, pp)
                t1 = rot.tile([128, SB], F16, tag="t1")
                nc.vector.tensor_mul(t1, q16, cosT[:, ts(sb, SB)])
                t2 = rot.tile([128, SB], F16, tag="t2")
                nc.vector.tensor_mul(t2[0:64], q16[64:128], sinF[64:128, ts(sb, SB)])
                nc.vector.tensor_mul(t2[64:128], q16[0:64], sinF[0:64, ts(sb, SB)])
                nc.vector.tensor_add(dst_slice, t1, t2)

            def emit_v(xv_s, sb):
                """V projection for slab sb (deferred one slab iteration so
                xv/wv bytes stay out of the startup DMA-critical window)."""
                for st in range(4):
                    pv = psum.tile([128, HPC * DH], F32, tag="zacc", bufs=2)
                    for dt in range(NDT):
                        nc.tensor.matmul(
                            pv,
                            lhsT=xv_s[:, ds(dt * SB + st * 128, 128)],
                            rhs=wv_sb[:, ts(dt, HPC * DH)],
                            start=(dt == 0), stop=(not with_bias and dt == NDT - 1),
                        )
                    if with_bias:
                        nc.tensor.matmul(pv, lhsT=ones128, rhs=bv_sb,
                                         start=False, stop=True)
                    if sb == NSB - 1:
                        nc.vector.tensor_copy(vsb[:, ts(sb * 4 + st, HPC * DH)], pv)
                    else:
                        nc.scalar.copy(vsb[:, ts(sb * 4 + st, HPC * DH)], pv)

            prev_v = None
            for sb in range(NSB):
                if sb == 0:
                    xq_s, xk_s, xv_s = xq_s0, xk_s0, xv_s0
                else:
                    xq_s = slabs.tile([128, NDT * SB], F16, tag="slab")
                    CH = NDT * SB // 2
                    for ci in range(2):
                        nc.sync.dma_start(
                            xq_s[:, ds(ci * CH, CH)], xq[sb][:, ds(ci * CH, CH)]
                        )
                    xk_s = slabs.tile([128, NDT * SB], F16, tag="slab")
                    nc.sync.dma_start(xk_s, xk[sb])
                    xv_s = slabs.tile([128, NDT * SB], F16, tag="slab")
                    nc.sync.dma_start(xv_s, xv[sb])
                    if sb == 2:
                        nc.sync.dma_start(wo_sb, wo[:])

                # All Q heads first, then all K heads: spreads each slab's
                # full-arrival deadline over ~16us instead of ~4us so the
                # DMA rings can keep up during the startup ramp.
                for h in range(HPC):
                    pq = psum.tile([128, SB], F32, tag="proj", bufs=2)
                    for dt in range(NDT):
                        nc.tensor.matmul(
                            pq,
                            lhsT=wq_sb[:, ds((h * NDT + dt) * DH, DH)],
                            rhs=xq_s[:, ts(dt, SB)],
                            start=(dt == 0), stop=(not with_bias and dt == NDT - 1),
                        )
                    if with_bias:
                        nc.tensor.matmul(pq, lhsT=bq_sb[:, ds(h * DH, DH)],
                                         rhs=ones512, start=False, stop=True)
                    rope_evac(pq, qT[:, ds(h * S + sb * SB, SB)], sb)

                for h in range(HPC):
                    pk = psum.tile([128, SB], F32, tag="scores", bufs=3)
                    for dt in range(NDT):
                        nc.tensor.matmul(
                            pk,
                            lhsT=wk_sb[:, ds((h * NDT + dt) * DH, DH)],
                            rhs=xk_s[:, ts(dt, SB)],
                            start=(dt == 0), stop=(not with_bias and dt == NDT - 1),
                        )
                    if with_bias:
                        nc.tensor.matmul(pk, lhsT=bk_sb[:, ds(h * DH, DH)],
                                         rhs=ones512, start=False, stop=True)
                    rope_evac(pk, kT[:, ds(h * S + sb * SB, SB)], sb)

                if prev_v is not None:
                    emit_v(*prev_v)
                prev_v = (xv_s, sb)
            emit_v(*prev_v)

        # ------- phases 2+3 interleaved: attention + output projection -------
        with tc.tile_pool(name="p2w", bufs=1) as p2w:

            def emit_p3(qb, st_lo=0, st_hi=4):
                for st in range(4 * qb + st_lo, 4 * qb + st_hi):
                    osb = p2w.tile([128, D], F16, tag="osb", bufs=2)
                    for db in range(4):
                        po = psum.tile([128, SB], F32, tag="proj", bufs=2)
                        for h in range(HPC):
                            nc.tensor.matmul(
                                po,
                                lhsT=zT[:, ds(h * S + st * 128, 128)],
                                rhs=wo_sb[:, ds(h * D + db * SB, SB)],
                                start=(h == 0), stop=(h == HPC - 1),
                            )
                        nc.vector.tensor_copy(osb[:, ts(db, SB)], po)
                        nc.sync.dma_start(out_part[ds(st * 128, 128), ts(db, SB)],
                                          osb[:, ts(db, SB)])

            # The per-head finalize (row-sum matmul -> reciprocal -> normalize)
            # is software-pipelined one head behind: the row-sum matmul waits
            # on the DVE pacc-add chain, so issuing it in-line stalls the PE
            # at every head boundary. Deferring it until the next head's kt=1
            # lets its DVE dependencies drain behind independent PE work.
            pending_fin = None

            for qb in range(NSB):
                nkt = 4 * qb + 4
                for h in range(HPC):
                    pz = psum.tile([128, SB], F32, tag="zacc", bufs=2)
                    pacc = p2w.tile([128, SB], F16, tag="pacc", bufs=3)
                    for kt in range(nkt):
                        t = kt - 4 * qb
                        q_lo = 0 if t < 0 else 128 * t
                        pscr = psum.tile([128, SB], F32, tag="scores", bufs=3)
                        nc.tensor.matmul(
                            pscr[:, q_lo:SB],
                            lhsT=kT[:, ds(h * S + kt * 128, 128)],
                            rhs=qT[:, ds(h * S + qb * SB + q_lo, SB - q_lo)],
                            start=True, stop=True,
                        )
                        pt = p2w.tile([128, SB], F16, tag="pt", bufs=8)
                        nc.scalar.activation(
                            pt[:, q_lo:SB], pscr[:, q_lo:SB],
                            mybir.ActivationFunctionType.Exp, scale=SCALE,
                        )
                        if t >= 0:
                            nc.vector.tensor_mul(
                                pt[:, ds(q_lo, 128)], pt[:, ds(q_lo, 128)], mask
                            )
                        nc.tensor.matmul(
                            pz[:, q_lo:SB],
                            lhsT=vsb[:, ds(kt * HPC * DH + h * DH, DH)],
                            rhs=pt[:, q_lo:SB],
                            start=(kt == 0), stop=(kt == nkt - 1),
                        )
                        # batched row-sum accumulation on the DVE
                        if kt == 0:
                            nc.vector.tensor_copy(pacc, pt)
                        else:
                            nc.vector.tensor_add(
                                pacc[:, q_lo:SB], pacc[:, q_lo:SB], pt[:, q_lo:SB]
                            )
                        if kt == 1 and pending_fin is not None:
                            pending_fin()
                            pending_fin = None

                    def _fin(pz=pz, pacc=pacc, h=h, qb=qb):
                        pr = psum.tile([128, SB], F32, tag="rsum", bufs=1)
                        nc.tensor.matmul(pr, lhsT=ones_sq, rhs=pacc,
                                         start=True, stop=True)
                        rr = p2w.tile([128, SB], F32, tag="rr", bufs=2)
                        nc.vector.reciprocal_approx_fast(out=rr, in_=pr)
                        nc.vector.tensor_mul(
                            zT[:, ds(h * S + qb * SB, SB)], pz, rr
                        )
                    pending_fin = _fin

                # Emission lags one q-block so the scheduler can fill exp-
                # latency PE gaps with projection matmuls. qb3's stretch has
                # the most k-tiles, so a slice of qb1's emission is held back
                # to give it extra filler.
                if qb == 1:
                    emit_p3(0)
                elif qb == 2:
                    emit_p3(1, 0, 3)
                elif qb == 3:
                    emit_p3(1, 3, 4)
                    emit_p3(2)
            pending_fin()
            emit_p3(3)

    nc.compile()
    return nc


def _rotary_tables():
    pos = np.arange(S, dtype=np.float64)
    dim = np.arange(DH // 2, dtype=np.float64)
    freq = ROT_BASE ** (dim / (DH / 2))  # base ** (dim / 64)
    freq = np.concatenate([freq, freq])
    angles = pos[:, None] / freq[None, :]          # [S, 128]
    cos_t = np.cos(angles).T.astype(np.float16)    # [128, S]
    sin_t = np.sin(angles).T.astype(np.float16)
    # halves pre-swapped so each rotary mul reads equal base partitions:
    # rows 64:128 = -sin (multiplies q16[64:128] into out[0:64]),
    # rows 0:64   = +sin (multiplies q16[0:64]  into out[64:128])
    sin_f = np.concatenate([sin_t[64:], -sin_t[:64]], axis=0)
    return np.ascontiguousarray(cos_t), np.ascontiguousarray(sin_f)


def _x_slabs(x2d):
    """[S, D] fp32 -> [NSB, 128, NDT*SB] fp16 slab layout of X^T."""
    xt = x2d.T.astype(np.float16)                          # [D, S]
    return np.ascontiguousarray(
        xt.reshape(NDT, 128, NSB, SB).transpose(2, 1, 0, 3).reshape(NSB, 128, NDT * SB)
    )


def _prep_in_maps(inputs, with_bias):
    q_in = np.asarray(inputs["query_input"], np.float32)
    k_in = np.asarray(inputs["key_input"], np.float32)
    v_in = np.asarray(inputs["value_input"], np.float32)
    W_Q = np.asarray(inputs["W_Q"], np.float32)
    W_K = np.asarray(inputs["W_K"], np.float32)
    W_V = np.asarray(inputs["W_V"], np.float32)
    W_O = np.asarray(inputs["W_O"], np.float32)
    b_Q = np.asarray(inputs["b_Q"], np.float32)
    b_K = np.asarray(inputs["b_K"], np.float32)
    b_V = np.asarray(inputs["b_V"], np.float32)

    cos_t, sin_f = _rotary_tables()
    mask_tri = np.triu(np.ones((128, 128), np.float16))    # [k, q]: 1 where k <= q
    ones_sq = np.ones((128, 128), np.float16)

    xq_b = [_x_slabs(q_in[b]) for b in range(B)]
    xk_b = [_x_slabs(k_in[b]) for b in range(B)]
    xv_b = [_x_slabs(v_in[b]) for b in range(B)]

    def w_lhsT(Wg):  # [4, D, DH] -> [128, HPC*NDT*DH]
        return np.ascontiguousarray(
            Wg.reshape(HPC, NDT, 128, DH).transpose(2, 0, 1, 3).reshape(128, -1)
        ).astype(np.float16)

    def w_rhs_v(Wg):  # [4, D, DH] -> [128, NDT*HPC*DH]
        return np.ascontiguousarray(
            Wg.transpose(1, 0, 2).reshape(NDT, 128, HPC * DH)
            .transpose(1, 0, 2).reshape(128, -1)
        ).astype(np.float16)

    def w_rhs_o(Wg):  # [4, DH, D] -> [128, HPC*D]
        return np.ascontiguousarray(Wg.transpose(1, 0, 2).reshape(128, -1)).astype(
            np.float16
        )

    in_maps = []
    for c in range(NCORES):
        b, g = divmod(c, HPC)
        hs = slice(g * HPC, g * HPC + HPC)
        m = {
            "xq": xq_b[b], "xk": xk_b[b], "xv": xv_b[b],
            "wq": w_lhsT(W_Q[hs]), "wk": w_lhsT(W_K[hs]), "wv": w_rhs_v(W_V[hs]),
            "wo": w_rhs_o(W_O[hs]),
            "cos_t": cos_t, "sin_f": sin_f, "mask_tri": mask_tri,
            "ones_sq": ones_sq,
        }
        if with_bias:
            m.update({
                "bqf": b_Q[hs].reshape(1, -1).astype(np.float16),
                "bkf": b_K[hs].reshape(1, -1).astype(np.float16),
                "bvf": b_V[hs].reshape(1, -1).astype(np.float16),
                "ones512": np.ones((1, SB), np.float16),
                "ones128": np.ones((1, 128), np.float16),
            })
        in_maps.append(m)
    return in_maps


_NC_CACHE = {}


def _get_nc(with_bias=False):
    if with_bias not in _NC_CACHE:
        _NC_CACHE[with_bias] = _build_bass(with_bias)
    return _NC_CACHE[with_bias]


def run_sharded(inputs, trace=False, **kwargs):
    """Run the SPMD kernel; returns (full_output, BassKernelResults)."""
    with_bias = any(
        bool(np.any(np.asarray(inputs[k]))) for k in ("b_Q", "b_K", "b_V")
    )
    nc = _get_nc(with_bias)
    in_maps = _prep_in_maps(inputs, with_bias)
    res = run_bass_kernel_spmd(
        nc, in_maps, core_ids=list(range(NCORES)), trace=trace, **kwargs
    )
    b_O = np.asarray(inputs["b_O"], np.float32)
    full = np.zeros((B, S, D), np.float32)
    for c in range(NCORES):
        full[c // HPC] += np.asarray(res.results[c]["out_part"], np.float32)
    full += b_O[None, None, :]
    return full, res


def kernel(**inputs):
    full, _ = run_sharded(inputs, trace=False)
    return full
